# revision 12
# baseline (speedup 1.0000x reference)
"""Distributed GINE GNN kernel for 8 Trainium2 NeuronCores.

Sharding: nodes partitioned contiguously across cores (12500/core, padded to
12544 = 98 windows of 128); edges assigned to the core owning their dst;
src features read from a replicated bf16 copy of h, AllGather'd per layer.

Edge slots are ordered slab-major: grouped by (src slab of 25088 table rows,
dst window of 128 nodes), chunk counts maxed across cores so all 8 cores run
one SPMD program. h[src] is fetched with batched dma_gather (InstDMAGatherAnt,
int16 in-slab indices, 4096 rows per call) instead of per-chunk indirect
DMAs — SWDGE descriptor generation on the Pool engine is the scarce resource
(994 ns fixed per DMA instruction + 0.34 ns/descriptor).

Per 128-edge chunk:
  e   = [attr|1] @ [We;be]          (PE, K=17)
  e  += h_gathered                  (PE identity-matmul accumulate in PSUM)
  m   = relu(e)                     (ACT: fused relu on PSUM->SBUF eviction)
  agg[dst] += m                     (PE one-hot matmul into a per-(slab,window)
                                     PSUM accumulator; evicted to SBUF agg by
                                     ACT copy on first visit, DVE add after)
One-hot tiles are built on DVE via tensor_scalar is_equal against an iota row.
Node MLP runs D-major (BN folded into W1); PE transposes convert between
node-major and D-major. Pooling = one-hot matmuls by graph id (fp32 PSUM),
assembled into a global buffer by indirect row scatter, AllReduce'd; the
classifier runs redundantly on every core.
"""

import numpy as np
import ml_dtypes

import concourse.bass as bass
import concourse.bacc as bacc
import concourse.mybir as mybir
import concourse.tile as tile
from concourse import bass_utils

# ---------------- problem constants ----------------
N = 100000
E = 1600000
D = 128
ED = 16
L = 3
G = 1024
C = 10
BN_EPS = 1e-5

CORES = 8
NPC = N // CORES          # 12500
NLOC = 12544              # 98 * 128
NW = NLOC // 128          # 98 dst windows
HROWS = CORES * NLOC      # 100352 rows of replicated h
NSLAB = 4
SLAB = HROWS // NSLAB     # 25088 (fits int16 dma_gather indices)

GLW = 256                 # local graph-id window span (2 psum tiles of 128)
GTC = 32                  # chunks per dma_gather call (4096 rows)
SUB = 4                   # chunks per psum e-bank

BF16 = mybir.dt.bfloat16
F32 = mybir.dt.float32
I16 = mybir.dt.int16
I32 = mybir.dt.int32

bf16 = ml_dtypes.bfloat16


# ---------------- host-side prep ----------------

def prepare(x, edge_attr, We, be, W1, b1, gamma, beta, W2, b2,
            Wc1, bc1, Wc2, bc2, edge_index, batch):
    x = np.asarray(x, np.float32)
    edge_attr = np.asarray(edge_attr, np.float32)
    edge_index = np.asarray(edge_index, np.int64)
    batch = np.asarray(batch, np.int64)

    rstd = 1.0 / np.sqrt(1.0 + BN_EPS)
    s = rstd * np.asarray(gamma, np.float32)
    W1f = np.asarray(W1, np.float32) * s[:, None, :]
    b1f = np.asarray(b1, np.float32) * s + np.asarray(beta, np.float32)

    src, dst = edge_index[0], edge_index[1]
    sc = src // NPC
    src_row = sc * NLOC + (src - sc * NPC)          # HROWS row space
    slab = (src_row // SLAB).astype(np.int64)
    core = dst // NPC
    dstl = dst - core * NPC
    w_of = dstl // 128
    drel = (dstl % 128).astype(np.float32)

    # bucket edges by (core, slab, window); chunk counts maxed across cores
    NB = NSLAB * NW
    key = (core * NSLAB + slab) * NW + w_of
    order = np.argsort(key, kind="stable")
    key_s = key[order]
    bounds = np.searchsorted(key_s, np.arange(CORES * NB + 1))
    counts = np.diff(bounds).reshape(CORES, NSLAB, NW)
    chunks_sw = ((counts.max(axis=0) + 127) // 128).astype(np.int64)  # [NSLAB, NW]

    # runs (s, w, q0, q1) in slab-major order; gather calls tile each slab
    runs = []
    calls = []
    q = 0
    for s_ in range(NSLAB):
        slab_q0 = q
        for w_ in range(NW):
            ch = int(chunks_sw[s_, w_])
            if ch:
                runs.append((s_, w_, q, q + ch))
                q += ch
        for k in range(slab_q0, q, GTC):
            calls.append((s_, k, min(k + GTC, q)))
    NCH2 = q
    EC2 = NCH2 * 128
    call_of_chunk = np.zeros(NCH2, np.int64)
    for ci, (_, a, b) in enumerate(calls):
        call_of_chunk[a:b] = ci

    # slot base per (slab, window) bucket, shared by all cores
    slot_base = np.zeros(NB, np.int64)
    for (s_, w_, q0, _) in runs:
        slot_base[s_ * NW + w_] = q0 * 128

    rank = np.arange(E) - bounds[key_s]
    slot = slot_base[key_s % NB] + rank
    core_s = key_s // NB

    idx16 = np.zeros((CORES, EC2), np.int16)
    attr_s = np.zeros((CORES, EC2, ED), np.float32)
    drel_s = np.full((CORES, EC2), -1.0, np.float32)
    idx16[core_s, slot] = (src_row[order] - slab[order] * SLAB).astype(np.int16)
    attr_s[core_s, slot] = edge_attr[order]
    drel_s[core_s, slot] = drel[order]

    # wrapped int16 indices: slot i -> (partition i%16, col i//16), replicated
    # across the 8 gpsimd-core partition groups
    idx_w = idx16.reshape(CORES, EC2 // 16, 16).transpose(0, 2, 1)
    idx_w = np.ascontiguousarray(np.tile(idx_w, (1, 8, 1)))

    attrT = np.ones((CORES, ED + 1, EC2), np.float32)
    attrT[:, :ED, :] = np.transpose(attr_s, (0, 2, 1))
    attrT = attrT.astype(bf16)
    dstrel_w = np.ascontiguousarray(
        drel_s.reshape(CORES, NCH2, 128).transpose(0, 2, 1))

    # first slab visiting each window (for copy-vs-add eviction)
    first_visit = {}
    for (s_, w_, _, _) in runs:
        first_visit.setdefault(w_, s_)
    layout = dict(runs=tuple(runs), calls=tuple(calls),
                  call_of_chunk=tuple(call_of_chunk.tolist()),
                  first_visit=tuple(sorted(first_visit.items())),
                  NCH2=NCH2, EC2=EC2)

    # x: replicated storage rows + per-core node-major wrapped local panel
    x_full = np.zeros((HROWS, D), np.float32)
    x_locN = np.zeros((CORES, 128, NLOC), np.float32)
    for c in range(CORES):
        xc = x[c * NPC:(c + 1) * NPC]
        x_full[c * NLOC: c * NLOC + NPC] = xc
        xp = np.zeros((NLOC, D), np.float32)
        xp[:NPC] = xc
        # node i at partition i%128, cols (i//128)*128 : +128
        x_locN[c] = xp.reshape(NW, 128, D).transpose(1, 0, 2).reshape(128, NLOC)
    x_full = x_full.astype(bf16)
    x_locN = x_locN.astype(bf16)

    # pooling: glocal[p, w] = batch[local node w*128+p] - gbase (pad -> -1)
    glocal = np.full((CORES, 128, NW), -1, np.float32)
    pool_rows = np.zeros((CORES, 128, 2), np.int32)
    for c in range(CORES):
        bb = batch[c * NPC:(c + 1) * NPC]
        gb = int(bb[0])
        span = int(bb[-1] - bb[0])
        assert span < GLW, f"graph span {span} exceeds {GLW}"
        gl = np.full(NLOC, -1, np.int64)
        gl[:NPC] = bb - gb
        glocal[c] = gl.reshape(NW, 128).T.astype(np.float32)
        pool_rows[c, :, 0] = gb + np.arange(128)
        pool_rows[c, :, 1] = gb + 128 + np.arange(128)
    pool_rows = np.clip(pool_rows, 0, G + GLW - 1).astype(np.int32)

    weights = dict(
        WeT=np.ascontiguousarray(np.asarray(We, np.float32)).astype(bf16),
        beb=np.asarray(be, np.float32).astype(bf16),
        W1f=W1f.astype(bf16), W2=np.asarray(W2, np.float32).astype(bf16),
        b1f=b1f.astype(np.float32), b2=np.asarray(b2, np.float32),
        Wc1=np.asarray(Wc1, np.float32).astype(bf16),
        Wc2=np.asarray(Wc2, np.float32).astype(bf16),
        bc1=np.asarray(bc1, np.float32), bc2=np.asarray(bc2, np.float32),
    )
    aux = dict(
        iota=np.tile(np.arange(128, dtype=np.float32), (128, 1)),
        iota2=np.tile(np.arange(GLW, dtype=np.float32), (128, 1)),
        ident=np.eye(128, dtype=np.float32).astype(bf16),
    )
    return dict(layout=layout, idx_w=idx_w, dstrel=dstrel_w, attrT=attrT,
                x_full=x_full, x_locN=x_locN, glocal=glocal,
                pool_rows=pool_rows, weights=weights, aux=aux)


# ---------------- device program ----------------

def build_program(layout, repeats=1):
    nc = bacc.Bacc("TRN2", target_bir_lowering=False, debug=False,
                   num_devices=CORES, num_swdge_queues=4)
    runs = layout["runs"]
    calls = layout["calls"]
    call_of_chunk = layout["call_of_chunk"]
    first_visit = dict(layout["first_visit"])
    NCH2 = layout["NCH2"]
    EC2 = layout["EC2"]

    t_xfull = nc.dram_tensor("x_full", [HROWS, D], BF16, kind="ExternalInput")
    t_xlocN = nc.dram_tensor("x_locN", [128, NLOC], BF16, kind="ExternalInput")
    t_idx = nc.dram_tensor("idx_w", [128, EC2 // 16], I16, kind="ExternalInput")
    t_dstrel = nc.dram_tensor("dstrel", [128, NCH2], F32, kind="ExternalInput")
    t_attrT = nc.dram_tensor("attrT", [ED + 1, EC2], BF16, kind="ExternalInput")
    t_glocal = nc.dram_tensor("glocal", [128, NW], F32, kind="ExternalInput")
    t_prows = nc.dram_tensor("pool_rows", [128, 2], I32, kind="ExternalInput")
    t_iota = nc.dram_tensor("iota", [128, 128], F32, kind="ExternalInput")
    t_iota2 = nc.dram_tensor("iota2", [128, GLW], F32, kind="ExternalInput")
    t_ident = nc.dram_tensor("ident", [128, 128], BF16, kind="ExternalInput")
    t_WeT = nc.dram_tensor("WeT", [L, ED, D], BF16, kind="ExternalInput")
    t_beb = nc.dram_tensor("beb", [L, D], BF16, kind="ExternalInput")
    t_W1f = nc.dram_tensor("W1f", [L, D, D], BF16, kind="ExternalInput")
    t_W2 = nc.dram_tensor("W2", [L, D, D], BF16, kind="ExternalInput")
    t_b1f = nc.dram_tensor("b1f", [L, D], F32, kind="ExternalInput")
    t_b2 = nc.dram_tensor("b2", [L, D], F32, kind="ExternalInput")
    t_Wc1 = nc.dram_tensor("Wc1", [D, D], BF16, kind="ExternalInput")
    t_Wc2 = nc.dram_tensor("Wc2", [D, C], BF16, kind="ExternalInput")
    t_bc1 = nc.dram_tensor("bc1", [D], F32, kind="ExternalInput")
    t_bc2 = nc.dram_tensor("bc2", [C], F32, kind="ExternalInput")

    t_out = nc.dram_tensor("out", [C, G], F32, kind="ExternalOutput")

    GROWS = G + GLW

    with tile.TileContext(nc) as tc:
        with (
            tc.tile_pool(name="persist", bufs=1) as pp,
            tc.tile_pool(name="edges", bufs=3) as ep,
            tc.tile_pool(name="msg", bufs=3) as mp,
            tc.tile_pool(name="small", bufs=4) as sp,
            tc.tile_pool(name="nodes", bufs=1) as np1,
            tc.tile_pool(name="pse", bufs=2, space="PSUM") as pse,
            tc.tile_pool(name="psa", bufs=2, space="PSUM") as psa,
            tc.tile_pool(name="psn", bufs=2, space="PSUM") as psn,
            tc.tile_pool(name="dram", bufs=1, space="DRAM") as dp,
        ):
            # ---- persistent tiles ----
            h_pan = pp.tile([128, NLOC], BF16, tag="h")          # node-major
            nc.sync.dma_start(h_pan[:], t_xlocN[:])
            idx_t = pp.tile([128, EC2 // 16], I16, tag="idx")
            nc.sync.dma_start(idx_t[:], t_idx[:])
            dstrel_t = pp.tile([128, NCH2], F32, tag="dstrel")
            nc.sync.dma_start(dstrel_t[:], t_dstrel[:])
            glocal_t = pp.tile([128, NW], F32, tag="glocal")
            nc.sync.dma_start(glocal_t[:], t_glocal[:])
            prows_t = pp.tile([128, 2], I32, tag="prows")
            nc.sync.dma_start(prows_t[:], t_prows[:])
            iota_t = pp.tile([128, 128], F32, tag="iota")
            nc.sync.dma_start(iota_t[:], t_iota[:])
            iota2_t = pp.tile([128, GLW], F32, tag="iota2")
            nc.sync.dma_start(iota2_t[:], t_iota2[:])
            ident = pp.tile([128, 128], BF16, tag="ident")
            nc.sync.dma_start(ident[:], t_ident[:])

            biases = pp.tile([128, 8], F32, tag="biases")
            for l in range(L):
                nc.sync.dma_start(biases[:, 2 * l:2 * l + 1], t_b1f[l, :, None])
                nc.sync.dma_start(biases[:, 2 * l + 1:2 * l + 2], t_b2[l, :, None])
            nc.sync.dma_start(biases[:, 6:7], t_bc1[:, None])
            nc.sync.dma_start(biases[:C, 7:8], t_bc2[:, None])

            WCOLS = 3 * L * D + D + C
            wts = pp.tile([128, WCOLS], BF16, tag="wts")
            nc.vector.memset(wts[:], 0.0)
            for l in range(L):
                nc.sync.dma_start(wts[:ED, 3 * l * D:3 * l * D + D], t_WeT[l])
                nc.sync.dma_start(wts[ED:ED + 1, 3 * l * D:3 * l * D + D],
                                  t_beb[l, None, :])
                nc.sync.dma_start(wts[:, 3 * l * D + D:3 * l * D + 2 * D], t_W1f[l])
                nc.sync.dma_start(wts[:, 3 * l * D + 2 * D:3 * l * D + 3 * D], t_W2[l])
            nc.sync.dma_start(wts[:, 3 * L * D:3 * L * D + D], t_Wc1[:])
            nc.sync.dma_start(wts[:, 3 * L * D + D:3 * L * D + D + C], t_Wc2[:])

            agg = pp.tile([128, NLOC], BF16, tag="agg")          # node-major

            ag_in = dp.tile([NLOC, D], BF16, tag="ag_in")
            ag_out = []
            for i in range(2):
                ago = dp.tile([HROWS, D], BF16, tag=f"ag_out{i}", name=f"ag_out{i}")
                ag_out.append(ago)
            pool_dram = dp.tile([GROWS, D], F32, tag="pool_dram")
            pool_red = dp.tile([GROWS, D], F32, tag="pool_red")

            def edge_layer(l, h_src_dram):
                wcol = 3 * l * D
                hg_t = {}
                at_t = {}
                state = {"next": 0}

                def issue_call(ci):
                    s_, a, b = calls[ci]
                    ncnk = b - a
                    hg = ep.tile([128, GTC, D], BF16, tag="hg")
                    nc.gpsimd.dma_gather(
                        out_ap=hg[:, :ncnk, :],
                        in_ap=h_src_dram[s_ * SLAB:, :],
                        idxs_ap=idx_t[:, a * 8:b * 8],
                        num_idxs=ncnk * 128,
                        num_idxs_reg=ncnk * 128,
                        elem_size=D,
                        single_packet=False,
                        queue_num=ci % 4,
                    )
                    at = ep.tile([ED + 1, GTC * 128], BF16, tag="attrT")
                    nc.sync.dma_start(at[:, :ncnk * 128],
                                      t_attrT[:, a * 128:b * 128])
                    hg_t[ci] = hg
                    at_t[ci] = at
                    if ci - 3 in hg_t:
                        del hg_t[ci - 3], at_t[ci - 3]

                def ensure(ci):
                    while state["next"] <= ci and state["next"] < len(calls):
                        issue_call(state["next"])
                        state["next"] += 1

                for (s_, w_, q0, q1) in runs:
                    ensure(min(call_of_chunk[q1 - 1] + 1, len(calls) - 1))
                    apsum = psa.tile([128, D], F32, tag="apsum")
                    for g0 in range(q0, q1, SUB):
                        g1 = min(g0 + SUB, q1)
                        ng = g1 - g0
                        eps = pse.tile([128, SUB * D], F32, tag="eps")
                        for q_ in range(g0, g1):
                            ci = call_of_chunk[q_]
                            a = calls[ci][1]
                            j = q_ - g0
                            nc.tensor.matmul(
                                eps[:, j * D:(j + 1) * D],
                                at_t[ci][:, (q_ - a) * 128:(q_ - a + 1) * 128],
                                wts[:ED + 1, wcol:wcol + D],
                                start=(q_ == g0), stop=False)
                        for q_ in range(g0, g1):
                            ci = call_of_chunk[q_]
                            a = calls[ci][1]
                            j = q_ - g0
                            nc.tensor.matmul(
                                eps[:, j * D:(j + 1) * D],
                                ident[:], hg_t[ci][:, q_ - a, :],
                                start=False, stop=(q_ == g1 - 1))
                        m = mp.tile([128, SUB, D], BF16, tag="m")
                        nc.scalar.activation(
                            m[:, :ng, :].rearrange("p a d -> p (a d)"),
                            eps[:, :ng * D],
                            mybir.ActivationFunctionType.Relu)
                        for q_ in range(g0, g1):
                            j = q_ - g0
                            oh = sp.tile([128, 128], BF16, tag="oh")
                            nc.vector.tensor_scalar(
                                oh[:], iota_t[:], dstrel_t[:, q_:q_ + 1], None,
                                op0=mybir.AluOpType.is_equal)
                            nc.tensor.matmul(
                                apsum[:], oh[:], m[:, j, :],
                                start=(q_ == q0), stop=(q_ == q1 - 1))
                    wc = agg[:, w_ * 128:(w_ + 1) * 128]
                    if first_visit[w_] == s_:
                        nc.scalar.copy(wc, apsum[:])
                    else:
                        nc.vector.tensor_add(wc, wc, apsum[:])

            def node_mlp(l):
                # z = h + agg, written into agg (agg is dead until next layer)
                nc.vector.tensor_add(agg[:], h_pan[:], agg[:])
                zD = np1.tile([128, NLOC], BF16, tag="zD")
                for w in range(NW):
                    tp = psn.tile([128, 128], BF16, tag="nps", name="tp")
                    nc.tensor.transpose(tp[:], agg[:, w * 128:(w + 1) * 128], ident[:])
                    nc.vector.tensor_copy(zD[:, w * 128:(w + 1) * 128], tp[:])
                spans = [(i * 512, 512) for i in range(NLOC // 512)]
                if NLOC % 512:
                    spans.append((NLOC - NLOC % 512, NLOC % 512))
                for (o, wd) in spans:
                    ps = psn.tile([128, 512], F32, tag="nps")
                    nc.tensor.matmul(ps[:, :wd],
                                     wts[:, 3 * l * D + D:3 * l * D + 2 * D],
                                     zD[:, o:o + wd], start=True, stop=True)
                    nc.scalar.activation(zD[:, o:o + wd], ps[:, :wd],
                                         mybir.ActivationFunctionType.Relu,
                                         bias=biases[:, 2 * l:2 * l + 1])
                for (o, wd) in spans:
                    ps = psn.tile([128, 512], F32, tag="nps")
                    nc.tensor.matmul(ps[:, :wd],
                                     wts[:, 3 * l * D + 2 * D:3 * l * D + 3 * D],
                                     zD[:, o:o + wd], start=True, stop=True)
                    nc.scalar.activation(zD[:, o:o + wd], ps[:, :wd],
                                         mybir.ActivationFunctionType.Relu,
                                         bias=biases[:, 2 * l + 1:2 * l + 2])
                # back to node-major h
                for w in range(NW):
                    tp = psn.tile([128, 128], BF16, tag="nps", name="tp")
                    nc.tensor.transpose(tp[:], zD[:, w * 128:(w + 1) * 128], ident[:])
                    nc.vector.tensor_copy(h_pan[:, w * 128:(w + 1) * 128], tp[:])

            # ---------------- layers ----------------
            # repeats>1 builds a timing variant: the layer loop runs
            # repeats times so per-iteration cost can be measured precisely
            # as (wall_R - wall_1) / (R - 1).
            TLAY = L * repeats
            for i in range(TLAY):
                l = i % L
                h_src = t_xfull[:] if i == 0 else ag_out[(i - 1) % 2][:]
                edge_layer(l, h_src)
                node_mlp(l)
                if i < TLAY - 1:
                    nc.sync.dma_start(
                        ag_in[:].rearrange("(b p) d -> p b d", p=128),
                        h_pan[:].rearrange("p (b d) -> p b d", d=D))
                    nc.gpsimd.collective_compute(
                        "AllGather", mybir.AluOpType.bypass,
                        ins=[ag_in.opt()], outs=[ag_out[i % 2].opt()],
                        replica_groups=[list(range(CORES))])

            # ---------------- pooling ----------------
            pps = psa.tile([128, 2, D], F32, tag="pps")
            pps0 = pps[:, 0, :]
            pps1 = pps[:, 1, :]
            for w in range(NW):
                oh0 = sp.tile([128, 128], BF16, tag="oh")
                nc.vector.tensor_scalar(
                    oh0[:], iota2_t[:, :128], glocal_t[:, w:w + 1], None,
                    op0=mybir.AluOpType.is_equal)
                nc.tensor.matmul(pps0, oh0[:], h_pan[:, w * 128:(w + 1) * 128],
                                 start=(w == 0), stop=False)
                oh1 = sp.tile([128, 128], BF16, tag="oh")
                nc.vector.tensor_scalar(
                    oh1[:], iota2_t[:, 128:], glocal_t[:, w:w + 1], None,
                    op0=mybir.AluOpType.is_equal)
                nc.tensor.matmul(pps1, oh1[:], h_pan[:, w * 128:(w + 1) * 128],
                                 start=False, stop=(w == NW - 1))
            pool_sb = np1.tile([128, 2, D], F32, tag="pool_sb")
            nc.scalar.copy(pool_sb[:, 0, :], pps0)
            nc.scalar.copy(pool_sb[:, 1, :], pps1)

            # zero the global pooled buffer, then place partials at gbase rows
            zt = np1.tile([128, (GROWS // 128) * D], F32, tag="zt")
            nc.vector.memset(zt[:], 0.0)
            nc.sync.dma_start(
                pool_dram[:].rearrange("(a p) d -> p a d", p=128),
                zt[:].rearrange("p (a d) -> p a d", d=D))
            for i in range(2):
                nc.gpsimd.indirect_dma_start(
                    out=pool_dram[:], out_offset=bass.IndirectOffsetOnAxis(
                        ap=prows_t[:, i:i + 1], axis=0),
                    in_=pool_sb[:, i, :], in_offset=None)
            nc.gpsimd.collective_compute(
                "AllReduce", mybir.AluOpType.add,
                ins=[pool_dram.opt()], outs=[pool_red.opt()],
                replica_groups=[list(range(CORES))])

            # ---------------- classifier ----------------
            prows_n = np1.tile([128, G // 128, D], F32, tag="prows_n")
            nc.sync.dma_start(
                prows_n[:], pool_red[:G, :].rearrange("(b p) d -> p b d", p=128))
            prows_bf = np1.tile([128, G // 128, D], BF16, tag="prows_bf")
            nc.vector.tensor_copy(
                prows_bf[:].rearrange("p a d -> p (a d)"),
                prows_n[:].rearrange("p a d -> p (a d)"))
            pooled_bf = np1.tile([128, G], BF16, tag="pooled_bf")   # D-major
            for b in range(G // 128):
                tp = psn.tile([128, 128], BF16, tag="nps", name="tp")
                nc.tensor.transpose(tp[:], prows_bf[:, b, :], ident[:])
                nc.vector.tensor_copy(pooled_bf[:, b * 128:(b + 1) * 128], tp[:])
            q1 = np1.tile([128, G], BF16, tag="q1")
            for o in range(0, G, 512):
                wd = min(512, G - o)
                ps = psn.tile([128, 512], F32, tag="nps")
                nc.tensor.matmul(ps[:, :wd], wts[:, 3 * L * D:3 * L * D + D],
                                 pooled_bf[:, o:o + wd], start=True, stop=True)
                nc.scalar.activation(q1[:, o:o + wd], ps[:, :wd],
                                     mybir.ActivationFunctionType.Relu,
                                     bias=biases[:, 6:7])
            outt = np1.tile([C, G], F32, tag="outt")
            for o in range(0, G, 512):
                wd = min(512, G - o)
                ps = psn.tile([128, 512], F32, tag="nps")
                nc.tensor.matmul(ps[:C, :wd], wts[:, 3 * L * D + D:3 * L * D + D + C],
                                 q1[:, o:o + wd], start=True, stop=True)
                nc.scalar.activation(outt[:, o:o + wd], ps[:C, :wd],
                                     mybir.ActivationFunctionType.Identity,
                                     bias=biases[:C, 7:8])
            nc.sync.dma_start(t_out[:], outt[:])

    nc.compile()
    return nc


_PROGRAM_CACHE = {}


def _get_program(layout):
    key = (layout["runs"], layout["calls"])
    if key not in _PROGRAM_CACHE:
        _PROGRAM_CACHE[key] = build_program(layout)
    return _PROGRAM_CACHE[key]


def make_in_maps(prep):
    w = prep["weights"]
    a = prep["aux"]
    in_maps = []
    for c in range(CORES):
        in_maps.append({
            "x_full": prep["x_full"],
            "x_locN": np.ascontiguousarray(prep["x_locN"][c]),
            "idx_w": np.ascontiguousarray(prep["idx_w"][c]),
            "dstrel": np.ascontiguousarray(prep["dstrel"][c]),
            "attrT": np.ascontiguousarray(prep["attrT"][c]),
            "glocal": np.ascontiguousarray(prep["glocal"][c]),
            "pool_rows": np.ascontiguousarray(prep["pool_rows"][c]),
            "iota": a["iota"], "iota2": a["iota2"], "ident": a["ident"],
            "WeT": w["WeT"], "beb": w["beb"], "W1f": w["W1f"], "W2": w["W2"],
            "b1f": w["b1f"], "b2": w["b2"],
            "Wc1": w["Wc1"], "Wc2": w["Wc2"], "bc1": w["bc1"], "bc2": w["bc2"],
        })
    return in_maps


def postprocess(out):
    return np.ascontiguousarray(out.T.astype(np.float32))


def kernel(**inputs):
    prep = prepare(**{k: np.asarray(v) for k, v in inputs.items()})
    nc = _get_program(prep["layout"])
    res = bass_utils.run_bass_kernel_spmd(nc, make_in_maps(prep),
                                          core_ids=list(range(CORES)))
    return postprocess(res.results[0]["out"])
